# revision 1
# baseline (speedup 1.0000x reference)
"""Channel-attention module kernel for Trainium2 (8 NeuronCores, data parallel).

Computes, per batch b:
    flat   = x[b].reshape(C, H*W)
    scores = flat @ flat.T                       # [C, C]
    attn   = softmax(scores, axis=-1)
    attn   = max(attn, -1, keepdims) - attn
    e      = attn.T @ flat                       # [C, H*W]
    out[b] = x[b] + beta * e

Key identity used: with m = rowmax(scores), S = sum(exp(scores - m)),
    rowmax(softmax) - softmax = (1 - exp(scores - m)) / S
so attn (with beta folded in) = beta/S - (beta/S) * exp(scores - m).

The matmul path runs in bf16. Instead of a separate cast pass, matmul
operands read the high 16 bits of each fp32 SBUF word through a stride-2
bf16 access pattern (bf16 = fp32 truncated), so x is loaded exactly once.
The residual add reads the same fp32 x, keeping the x-contribution exact.

Sharding: batch dim (32) split over 8 cores, 4 batches per core, beta
replicated; no cross-core communication.
"""

import numpy as np

import concourse.bass as bass
import concourse.mybir as mybir
import concourse.tile as tile
from concourse import bacc
from concourse.bass_utils import run_bass_kernel_spmd
from concourse.masks import make_identity

N_CORES = 8
B_TOTAL, C, H, W = 32, 128, 128, 128
HW = H * W                      # 16384
B_LOCAL = B_TOTAL // N_CORES    # 4
P = 128

F32 = mybir.dt.float32
BF16 = mybir.dt.bfloat16

MM_N = 512                      # stage-2 matmul free dim (one PSUM bank fp32)
E_TILE = 1024                   # stage-2 psum tile (2 banks, 2 matmuls, 1 add)
TG = 8                          # transposed 128-chunks per bf16 PSUM bank
OUT_CHUNK = 2048                # output staging chunk (8 KB/partition, 1 MB DMA)
IN_CHUNK = 4096                 # input DMA chunk (2 MB DMA)
LOOKAHEAD = 3                   # transpose groups of batch b+1 emitted pre-S2(b)


def _bf16_high_view(ap_f32: bass.AP) -> bass.AP:
    """View the high 16 bits of each fp32 element as a stride-2 bf16 AP."""
    v = ap_f32.bitcast(BF16)                       # [P, 2*N]
    v = v.rearrange("p (n two) -> p n two", two=2)  # [P, N, 2]
    return v[:, :, 1]                               # little-endian high half


def build_bass(b_local: int = B_LOCAL) -> bass.Bass:
    nc = bacc.Bacc("TRN2", target_bir_lowering=False)
    x = nc.dram_tensor("x", [b_local, C, HW], F32, kind="ExternalInput")
    beta = nc.dram_tensor("beta", [1], F32, kind="ExternalInput")
    out = nc.dram_tensor("out", [b_local, C, HW], F32, kind="ExternalOutput")

    n_chunk = HW // P           # 128 transposed chunks per batch
    n_group = n_chunk // TG     # 16
    n_out = HW // OUT_CHUNK     # 8
    e_per_out = OUT_CHUNK // E_TILE
    mm_per_e = E_TILE // MM_N

    with tile.TileContext(nc) as tc:
        with (
            tc.tile_pool(name="singles", bufs=1) as singles,
            tc.tile_pool(name="flats", bufs=2) as flats,
            tc.tile_pool(name="ats", bufs=2 + LOOKAHEAD) as ats,
            tc.tile_pool(name="outs", bufs=3) as outs,
            tc.tile_pool(name="sm", bufs=2) as sm,
            tc.tile_pool(name="ps_t", bufs=3, space="PSUM") as ps_t,
            tc.tile_pool(name="ps_s", bufs=1, space="PSUM") as ps_s,
            tc.tile_pool(name="ps_e", bufs=2, space="PSUM") as ps_e,
        ):
            ident = singles.tile([P, P], BF16)
            make_identity(nc, ident)

            beta_b = singles.tile([P, 1], F32)
            bap = beta[:]
            beta_bcast = bass.AP(
                tensor=bap.tensor, offset=bap.offset, ap=[[0, P], [1, 1]]
            )
            nc.gpsimd.dma_start(out=beta_b, in_=beta_bcast)
            negbeta_b = singles.tile([P, 1], F32)
            nc.vector.tensor_scalar_mul(negbeta_b, beta_b, -1.0)

            flat_tiles: dict[int, bass.AP] = {}
            at_tiles: dict[tuple[int, int], bass.AP] = {}
            scores_tiles: dict[int, bass.AP] = {}

            def emit_in_quarter(b, q):
                if b not in flat_tiles:
                    flat_tiles[b] = flats.tile(
                        [P, HW], F32, tag="flat", name=f"flat{b}"
                    )
                sl = slice(q * IN_CHUNK, (q + 1) * IN_CHUNK)
                nc.sync.dma_start(out=flat_tiles[b][:, sl], in_=x[b, :, sl])

            def emit_in(b):
                for q in range(HW // IN_CHUNK):
                    emit_in_quarter(b, q)

            def emit_t_group(b, g):
                hi = _bf16_high_view(flat_tiles[b])
                tp = ps_t.tile([P, TG * P], BF16, tag="tp")
                for jj in range(TG):
                    k = g * TG + jj
                    nc.tensor.transpose(
                        tp[:, jj * P : (jj + 1) * P],
                        hi[:, k * P : (k + 1) * P],
                        ident,
                    )
                at = ats.tile([P, TG * P], BF16, tag="at")
                nc.scalar.copy(out=at, in_=tp)
                at_tiles[(b, g)] = at

            def emit_m_group(b, g):
                if g == 0:
                    scores_tiles[b] = ps_s.tile(
                        [P, P], F32, tag="scores", name=f"scores{b}"
                    )
                scores_ps = scores_tiles[b]
                at = at_tiles.pop((b, g))
                for jj in range(TG):
                    k = g * TG + jj
                    nc.tensor.matmul(
                        scores_ps,
                        at[:, jj * P : (jj + 1) * P],
                        at[:, jj * P : (jj + 1) * P],
                        start=(k == 0),
                        stop=(k == n_chunk - 1),
                    )

            for b in range(min(2, b_local)):
                emit_in(b)

            for b in range(b_local):
                # ---- stage 1: interleaved transpose/matmul groups ----
                # (the first LOOKAHEAD transpose groups of b>0 were already
                # emitted at the end of the previous iteration)
                start_g = LOOKAHEAD if b > 0 else 0
                for g in range(n_group + 1):
                    if start_g <= g < n_group:
                        emit_t_group(b, g)
                    if g >= 1:
                        emit_m_group(b, g - 1)

                # prefetch next-next batch input; lookahead transposes of b+1
                if b + 2 < b_local:
                    emit_in(b + 2)
                if b + 1 < b_local:
                    for g in range(LOOKAHEAD):
                        emit_t_group(b + 1, g)

                # ---- softmax transform: attn = beta/S - (beta/S)*exp(s-m) ----
                scores_ps = scores_tiles.pop(b)
                neg_max = sm.tile([P, 1], F32, tag="neg_max")
                nc.vector.reduce_max(
                    out=neg_max,
                    in_=scores_ps,
                    axis=mybir.AxisListType.X,
                    negate=True,
                )
                ex = sm.tile([P, P], F32, tag="ex")
                nc.scalar.activation(
                    out=ex,
                    in_=scores_ps,
                    func=mybir.ActivationFunctionType.Exp,
                    bias=neg_max,
                    scale=1.0,
                )
                sumexp = sm.tile([P, 1], F32, tag="sumexp")
                nc.vector.reduce_sum(
                    out=sumexp, in_=ex, axis=mybir.AxisListType.X
                )
                r = sm.tile([P, 1], F32, tag="r")
                nc.vector.reciprocal(r, sumexp)
                rb = sm.tile([P, 1], F32, tag="rb")
                nc.vector.tensor_mul(rb, r, beta_b)
                nrb = sm.tile([P, 1], F32, tag="nrb")
                nc.vector.tensor_mul(nrb, r, negbeta_b)
                attn = sm.tile([P, P], BF16, tag="attn")
                # out = Identity(ex * nrb + rb) = rb - rb*ex
                nc.scalar.activation(
                    out=attn,
                    in_=ex,
                    func=mybir.ActivationFunctionType.Identity,
                    bias=rb,
                    scale=nrb,
                )

                # ---- stage 2: e = attn.T @ x16 (bf16), out = x + e ----
                flat = flat_tiles.pop(b)
                hi = _bf16_high_view(flat)
                for jo in range(n_out):
                    oc = outs.tile([P, OUT_CHUNK], F32, tag="oc")
                    for je in range(e_per_out):
                        e_ps = ps_e.tile([P, E_TILE], F32, tag="e")
                        for jm in range(mm_per_e):
                            j = (jo * e_per_out + je) * mm_per_e + jm
                            nc.tensor.matmul(
                                e_ps[:, jm * MM_N : (jm + 1) * MM_N],
                                attn,
                                hi[:, j * MM_N : (j + 1) * MM_N],
                                start=True,
                                stop=True,
                            )
                        nc.vector.tensor_add(
                            out=oc[:, je * E_TILE : (je + 1) * E_TILE],
                            in0=e_ps,
                            in1=flat[
                                :,
                                jo * OUT_CHUNK + je * E_TILE : jo * OUT_CHUNK
                                + (je + 1) * E_TILE,
                            ],
                        )
                    nc.gpsimd.dma_start(
                        out=out[b, :, jo * OUT_CHUNK : (jo + 1) * OUT_CHUNK],
                        in_=oc,
                    )
    nc.compile()
    return nc


_NC_CACHE: dict[int, bass.Bass] = {}


def _get_nc(b_local: int = B_LOCAL) -> bass.Bass:
    if b_local not in _NC_CACHE:
        _NC_CACHE[b_local] = build_bass(b_local)
    return _NC_CACHE[b_local]


def _run(x: np.ndarray, beta: np.ndarray, trace: bool = False):
    x = np.ascontiguousarray(np.asarray(x), dtype=np.float32)
    beta = np.ascontiguousarray(np.asarray(beta), dtype=np.float32).reshape(1)
    xr = x.reshape(B_TOTAL, C, HW)
    in_maps = []
    for i in range(N_CORES):
        shard = np.ascontiguousarray(xr[i * B_LOCAL : (i + 1) * B_LOCAL])
        in_maps.append({"x": shard, "beta": beta})
    nc = _get_nc()
    res = run_bass_kernel_spmd(
        nc, in_maps, core_ids=list(range(N_CORES)), trace=trace
    )
    parts = [res.results[i]["out"] for i in range(N_CORES)]
    full = np.concatenate(parts, axis=0).reshape(B_TOTAL, C, H, W)
    return np.ascontiguousarray(full, dtype=np.float32), res


def kernel(x: np.ndarray, beta: np.ndarray) -> np.ndarray:
    out, _ = _run(x, beta, trace=False)
    return out


def kernel_traced(x: np.ndarray, beta: np.ndarray):
    """Like kernel() but also returns the BassKernelResults (with profile)."""
    return _run(x, beta, trace=True)

